# revision 7
# baseline (speedup 1.0000x reference)
"""Bass/Tile TRN2 kernel for nn_CPAMDec (CPAM cross-attention decoder).

Sharding: data-parallel over batch — 8 samples, one per NeuronCore.
All parameters are replicated; each core computes its full sample.

Host-side (parameter-only) preprocessing:
  - eval-mode BatchNorm affines folded into the adjacent 1x1-conv weights
  - the two chained fx convs fused into a single 512x512 matrix
  - weights pre-transposed into the PE's stationary (lhsT) layout

Device-side per core (C=512 as 4 chunks of 128 partitions, hw=5184 as
12 tiles of 432 = 6 rows of 72):
  P0: stream x/y tiles; DVE reduces pool partials; PE computes
      z2 = Wc@x + bc into a resident [512,5184] buffer (float32r).
  P1: pooling hierarchy (block sums 12x12 -> pools 6,3,2,1), encoders as
      swapped-operand matmuls (pooled stationary, enc weights streamed),
      linear 50x50 layers, fy transposed back to [c,50].
  P2: per tile: simT = fy^T@z2 -> PE transpose -> row softmax ->
      PE transpose back -> fout = fself^T@attT -> fup conv -> bias +
      residual -> DMA out.
"""

import sys

for _p in ("/opt/trn_rl_repo", "/root/.axon_site/_ro/trn_rl_repo"):
    if _p not in sys.path:
        sys.path.append(_p)

import numpy as np

import concourse.bacc as bacc
import concourse.bass as bass
import concourse.mybir as mybir
import concourse.tile as tile
from concourse.bass_utils import run_bass_kernel_spmd
from concourse.masks import make_identity

F32 = mybir.dt.float32
F32R = mybir.dt.float32r
AX = mybir.AxisListType
AF = mybir.ActivationFunctionType
ALU = mybir.AluOpType

B, C, H, W = 8, 512, 72, 72
HW = H * W            # 5184
KC, P = 4, 128        # channel chunks x partitions
NT, TW = 12, 432      # hw tiles: 12 x (6 rows of 72)
NSUB, SUB = 4, 108    # row-subblocks per tile for softmax
NPOOL = 50            # 1 + 4 + 9 + 36
EPS = 1e-5
S_OFF = (0, 1, 5, 14)
S_LEN = (1, 4, 9, 36)

_NC = None


def _emit(nc):
    xd = nc.dram_tensor("xd", [KC, P, HW], F32R, kind="ExternalInput")
    yd = nc.dram_tensor("yd", [KC, P, HW], F32R, kind="ExternalInput")
    wct = nc.dram_tensor("wct", [KC, P, C], F32R, kind="ExternalInput")
    wupt = nc.dram_tensor("wupt", [KC, P, C], F32R, kind="ExternalInput")
    bcd = nc.dram_tensor("bcd", [P, KC], F32, kind="ExternalInput")
    bupd = nc.dram_tensor("bupd", [P, KC], F32, kind="ExternalInput")
    wxt = nc.dram_tensor("wxt", [4, KC, P, C], F32R, kind="ExternalInput")
    wyt = nc.dram_tensor("wyt", [4, KC, P, C], F32R, kind="ExternalInput")
    bexd = nc.dram_tensor("bexd", [1, 4, C], F32R, kind="ExternalInput")
    beyd = nc.dram_tensor("beyd", [1, 4, C], F32R, kind="ExternalInput")
    lxtd = nc.dram_tensor("lxtd", [4, S_LEN[3], NPOOL], F32R,
                          kind="ExternalInput")
    lytd = nc.dram_tensor("lytd", [4, S_LEN[3], NPOOL], F32R,
                          kind="ExternalInput")
    bxd = nc.dram_tensor("bxd", [NPOOL, 1], F32, kind="ExternalInput")
    byd = nc.dram_tensor("byd", [NPOOL, 1], F32, kind="ExternalInput")
    outd = nc.dram_tensor("outd", [KC, P, HW], F32, kind="ExternalOutput")

    with tile.TileContext(nc) as tc:
        _body(nc, tc, xd, yd, wct, wupt, bcd, bupd, wxt, wyt, bexd, beyd,
              lxtd, lytd, bxd, byd, outd)
    nc.compile()
    return nc


def _body(nc, tc, xd, yd, wct, wupt, bcd, bupd, wxt, wyt, bexd, beyd,
          lxtd, lytd, bxd, byd, outd):
    from contextlib import ExitStack
    ctx = ExitStack()
    with ctx:
        consts = ctx.enter_context(tc.tile_pool(name="consts", bufs=1))
        z2p = ctx.enter_context(tc.tile_pool(name="z2p", bufs=1))
        poolp = ctx.enter_context(tc.tile_pool(name="poolp", bufs=1))
        stream = ctx.enter_context(tc.tile_pool(name="stream", bufs=3))
        encp = ctx.enter_context(tc.tile_pool(name="encp", bufs=1))
        wenc = ctx.enter_context(tc.tile_pool(name="wenc", bufs=3))
        attp = ctx.enter_context(tc.tile_pool(name="attp", bufs=2))
        foutp = ctx.enter_context(tc.tile_pool(name="foutp", bufs=2))
        outp = ctx.enter_context(tc.tile_pool(name="outp", bufs=2))
        ps_conv = ctx.enter_context(
            tc.tile_pool(name="ps_conv", bufs=3, space="PSUM"))
        ps_aux = ctx.enter_context(
            tc.tile_pool(name="ps_aux", bufs=1, space="PSUM"))

        # ---- constants ----
        ident = consts.tile([P, P], F32)
        make_identity(nc, ident)
        wct_sb = consts.tile([P, KC, C], F32R, tag="wct")
        nc.sync.dma_start(out=wct_sb, in_=wct.ap().rearrange("k p m -> p k m"))
        wupt_sb = consts.tile([P, KC, C], F32R, tag="wupt")
        nc.sync.dma_start(out=wupt_sb, in_=wupt.ap().rearrange("k p m -> p k m"))
        bc_sb = consts.tile([P, KC], F32, tag="bc")
        nc.sync.dma_start(out=bc_sb, in_=bcd.ap())
        bup_sb = consts.tile([P, KC], F32, tag="bup")
        nc.sync.dma_start(out=bup_sb, in_=bupd.ap())
        lxt_sb = consts.tile([S_LEN[3], 4, NPOOL], F32R, tag="lxt")
        nc.sync.dma_start(out=lxt_sb, in_=lxtd.ap().rearrange("s j k -> j s k"))
        lyt_sb = consts.tile([S_LEN[3], 4, NPOOL], F32R, tag="lyt")
        nc.sync.dma_start(out=lyt_sb, in_=lytd.ap().rearrange("s j k -> j s k"))
        bx_sb = consts.tile([NPOOL, 1], F32, tag="bx")
        nc.sync.dma_start(out=bx_sb, in_=bxd.ap())
        by_sb = consts.tile([NPOOL, 1], F32, tag="by")
        nc.sync.dma_start(out=by_sb, in_=byd.ap())
        bex_sb = consts.tile([1, 4, C], F32R, tag="bex")
        nc.sync.dma_start(out=bex_sb, in_=bexd.ap())
        bey_sb = consts.tile([1, 4, C], F32R, tag="bey")
        nc.sync.dma_start(out=bey_sb, in_=beyd.ap())
        ones_f32 = consts.tile([1, S_LEN[3]], F32, tag="ones_f32")
        nc.vector.memset(ones_f32, 1.0)
        ones_sb = consts.tile([1, S_LEN[3]], F32R, tag="ones")
        nc.vector.tensor_copy(ones_sb, ones_f32)

        # ---- persistent buffers ----
        z2_sb = z2p.tile([P, KC, HW], F32R)
        partx = poolp.tile([P, KC, NT, 6], F32, tag="partx")
        party = poolp.tile([P, KC, NT, 6], F32, tag="party")

        def pool_partial(t, xt, part):
            # block sums over (6 rows, 12 cols) -> [P, 6 w-blocks] per chunk
            for kc in range(KC):
                src = xt[:, kc, :].rearrange(
                    "p (lh wb wl) -> p wb lh wl", lh=6, wl=12)
                nc.vector.reduce_sum(part[:, kc, t, :], src, axis=AX.XY)

        # ---- P0: stream x & y tiles ----
        for t in range(NT):
            sl = slice(t * TW, (t + 1) * TW)
            yt = stream.tile([P, KC, TW], F32R, tag="xt")
            nc.sync.dma_start(
                out=yt, in_=yd.ap()[:, :, sl].rearrange("k p n -> p k n"))
            pool_partial(t, yt, party)

            xt = stream.tile([P, KC, TW], F32R, tag="xt")
            nc.sync.dma_start(
                out=xt, in_=xd.ap()[:, :, sl].rearrange("k p n -> p k n"))
            pool_partial(t, xt, partx)
            for mc in range(KC):
                zp = ps_conv.tile([P, TW], F32, tag="cps")
                for kc in range(KC):
                    nc.tensor.matmul(
                        zp, wct_sb[:, kc, mc * P:(mc + 1) * P], xt[:, kc, :],
                        start=(kc == 0), stop=(kc == KC - 1))
                nc.scalar.activation(z2_sb[:, mc, sl], zp, AF.Identity,
                                     bias=bc_sb[:, mc:mc + 1], scale=1.0)

        # ---- P1a: pooling hierarchy -> pooled [P, KC, 50] (f32r) ----
        def finish_pool(part, pooled):
            p6 = poolp.tile([P, KC, 6, 6], F32, tag="p6")
            s3 = poolp.tile([P, KC, 6, 3], F32, tag="s3")
            p3 = poolp.tile([P, KC, 3, 3], F32, tag="p3")
            s2 = poolp.tile([P, KC, 6, 2], F32, tag="s2")
            p2 = poolp.tile([P, KC, 2, 2], F32, tag="p2")
            p1 = poolp.tile([P, KC, 1], F32, tag="p1")
            for kc in range(KC):
                nc.vector.reduce_sum(
                    p6[:, kc], part[:, kc].rearrange(
                        "p (hh half) wb -> p hh wb half", half=2), axis=AX.X)
                nc.vector.reduce_sum(
                    s3[:, kc], p6[:, kc].rearrange(
                        "p hh (w3 wl) -> p hh w3 wl", wl=2), axis=AX.X)
                nc.vector.reduce_sum(
                    p3[:, kc], s3[:, kc].rearrange(
                        "p (h3 hl) w3 -> p h3 w3 hl", hl=2), axis=AX.X)
                nc.vector.reduce_sum(
                    s2[:, kc], p6[:, kc].rearrange(
                        "p hh (w2 wl) -> p hh w2 wl", wl=3), axis=AX.X)
                nc.vector.reduce_sum(
                    p2[:, kc], s2[:, kc].rearrange(
                        "p (h2 hl) w2 -> p h2 w2 hl", hl=3), axis=AX.X)
                nc.vector.reduce_sum(
                    p1[:, kc], p6[:, kc].rearrange("p a b -> p (a b)"),
                    axis=AX.X)
                nc.vector.tensor_scalar_mul(
                    pooled[:, kc, 0:1], p1[:, kc], 1.0 / 5184)
                nc.vector.tensor_scalar_mul(
                    pooled[:, kc, 1:5],
                    p2[:, kc].rearrange("p a b -> p (a b)"), 1.0 / 1296)
                nc.vector.tensor_scalar_mul(
                    pooled[:, kc, 5:14],
                    p3[:, kc].rearrange("p a b -> p (a b)"), 1.0 / 576)
                nc.vector.tensor_scalar_mul(
                    pooled[:, kc, 14:50],
                    p6[:, kc].rearrange("p a b -> p (a b)"), 1.0 / 144)

        pooledx = poolp.tile([P, KC, NPOOL], F32R, tag="pooledx")
        pooledy = poolp.tile([P, KC, NPOOL], F32R, tag="pooledy")
        finish_pool(partx, pooledx)
        finish_pool(party, pooledy)

        # ---- P1b: encoders (enc^T [50, C]) ----
        def encoder(pooled, wt_dram, be_sb, enc_t):
            for s in range(4):
                off, ln = S_OFF[s], S_LEN[s]
                ep = ps_aux.tile([S_LEN[3], C], F32, tag="encps")
                for kc in range(KC):
                    wtile = wenc.tile([P, C], F32R, tag="we")
                    nc.sync.dma_start(out=wtile, in_=wt_dram.ap()[s, kc])
                    nc.tensor.matmul(
                        ep[:ln, :], pooled[:, kc, off:off + ln], wtile,
                        start=(kc == 0), stop=False)
                # + bias via a K=1 matmul: ones[1,ln].T @ be[1,C]
                nc.tensor.matmul(ep[:ln, :], ones_sb[:, :ln], be_sb[0:1, s, :],
                                 start=False, stop=True)
                nc.vector.tensor_scalar_max(enc_t[s][:ln, :], ep[:ln, :], 0.0)

        encx_t = [encp.tile([S_LEN[s], C], F32R, tag=f"encx{s}",
                             name=f"encx{s}") for s in range(4)]
        ency_t = [encp.tile([S_LEN[s], C], F32R, tag=f"ency{s}",
                             name=f"ency{s}") for s in range(4)]
        encoder(pooledx, wxt, bex_sb, encx_t)
        encoder(pooledy, wyt, bey_sb, ency_t)

        # ---- P1c: linear layers; fy back to [c, 50] ----
        fselfT = encp.tile([NPOOL, C], F32R, tag="fselfT")
        fp = ps_aux.tile([NPOOL, C], F32, tag="encps")
        for s in range(4):
            nc.tensor.matmul(fp, lxt_sb[:S_LEN[s], s, :], encx_t[s],
                             start=(s == 0), stop=(s == 3))
        nc.scalar.activation(fselfT, fp, AF.Identity, bias=bx_sb, scale=1.0)

        fyt2 = encp.tile([NPOOL, C], F32, tag="fyt2")
        fp2 = ps_aux.tile([NPOOL, C], F32, tag="encps")
        for s in range(4):
            nc.tensor.matmul(fp2, lyt_sb[:S_LEN[s], s, :], ency_t[s],
                             start=(s == 0), stop=(s == 3))
        nc.scalar.activation(fyt2, fp2, AF.Identity, bias=by_sb, scale=1.0)

        fy_sb = encp.tile([P, KC, NPOOL], F32R, tag="fy")
        for mc in range(KC):
            tp = ps_aux.tile([P, NPOOL], F32, tag="fybt")
            nc.tensor.transpose(tp, fyt2[:, mc * P:(mc + 1) * P],
                                ident[:NPOOL, :NPOOL])
            nc.vector.tensor_copy(fy_sb[:, mc, :], tp)

        # ---- P2: attention + fup conv, per tile ----
        for t in range(NT):
            sl = slice(t * TW, (t + 1) * TW)
            sp = ps_aux.tile([NPOOL, TW], F32, tag="simp")
            for kc in range(KC):
                nc.tensor.matmul(sp, fy_sb[:, kc, :], z2_sb[:, kc, sl],
                                 start=(kc == 0), stop=(kc == KC - 1))
            simT_t = attp.tile([NPOOL, TW], F32, tag="simT")
            nc.scalar.copy(simT_t, sp)

            rp = ps_aux.tile([SUB, NSUB, NPOOL], F32, tag="strp")
            for j in range(NSUB):
                nc.tensor.transpose(rp[:, j, :],
                                    simT_t[:, j * SUB:(j + 1) * SUB],
                                    ident[:NPOOL, :NPOOL])
            att_t = attp.tile([SUB, NSUB, NPOOL], F32, tag="att")
            negmax = attp.tile([SUB, NSUB], F32, tag="negmax")
            sumexp = attp.tile([SUB, NSUB], F32, tag="sumexp")
            rec = attp.tile([SUB, NSUB], F32, tag="rec")
            for j in range(NSUB):
                nc.vector.reduce_max(negmax[:, j:j + 1], rp[:, j, :],
                                     axis=AX.X, negate=True)
                nc.scalar.activation(att_t[:, j, :], rp[:, j, :], AF.Exp,
                                     bias=negmax[:, j:j + 1], scale=1.0,
                                     accum_out=sumexp[:, j:j + 1])
            nc.vector.reciprocal(rec, sumexp)
            for j in range(NSUB):
                nc.vector.tensor_scalar_mul(att_t[:, j, :], att_t[:, j, :],
                                            rec[:, j:j + 1])

            ap_ = ps_aux.tile([NPOOL, TW], F32, tag="attps")
            for j in range(NSUB):
                nc.tensor.transpose(ap_[:, j * SUB:(j + 1) * SUB],
                                    att_t[:, j, :], ident[:SUB, :SUB])
            attT_t = attp.tile([NPOOL, TW], F32R, tag="attT")
            nc.vector.tensor_copy(attT_t, ap_)

            fout_t = foutp.tile([P, KC, TW], F32R, tag="fout")
            for mc in range(KC):
                fpp = ps_conv.tile([P, TW], F32, tag="cps")
                nc.tensor.matmul(fpp, fselfT[:, mc * P:(mc + 1) * P], attT_t,
                                 start=True, stop=True)
                nc.vector.tensor_copy(fout_t[:, mc, :], fpp)

            x2 = stream.tile([P, KC, TW], F32R, tag="xt")
            nc.sync.dma_start(
                out=x2, in_=xd.ap()[:, :, sl].rearrange("k p n -> p k n"))

            out_t = outp.tile([P, KC, TW], F32, tag="out")
            for mc in range(KC):
                op_ = ps_conv.tile([P, TW], F32, tag="cps")
                for kc in range(KC):
                    nc.tensor.matmul(
                        op_, wupt_sb[:, kc, mc * P:(mc + 1) * P],
                        fout_t[:, kc, :],
                        start=(kc == 0), stop=(kc == KC - 1))
                nc.scalar.activation(out_t[:, mc, :], op_, AF.Identity,
                                     bias=bup_sb[:, mc:mc + 1], scale=1.0)
                nc.gpsimd.tensor_tensor(out_t[:, mc, :], out_t[:, mc, :],
                                        x2[:, mc, :].bitcast(F32), ALU.add)
            nc.sync.dma_start(
                out=outd.ap()[:, :, sl].rearrange("k p n -> p k n"),
                in_=out_t)


def _split_lin(lw):
    # lin weight [50,50]; lhsT rows j split by pool scale -> [4, 36, 50]
    lt = lw.T.astype(np.float32)  # [j, k]
    out = np.zeros((4, S_LEN[3], NPOOL), np.float32)
    for s in range(4):
        out[s, :S_LEN[s]] = lt[S_OFF[s]:S_OFF[s] + S_LEN[s]]
    return out


def _bn_fold(bn):
    g, bt, m, v = [a.astype(np.float64) for a in bn]
    a = g / np.sqrt(v + EPS)
    return a, bt.astype(np.float64) - a * m


def _prep(inputs):
    """Host-side fold + shard. Returns list of 8 per-core input maps."""
    f = {k: np.asarray(v) for k, v in inputs.items()}

    a1, b1 = _bn_fold(f["fx_bn"][0])
    a2, b2 = _bn_fold(f["fx_bn"][1])
    W1 = f["fx_w"][0].astype(np.float64)
    W2 = f["fx_w"][1].astype(np.float64)
    Wc = (a2[:, None] * W2) @ (a1[:, None] * W1)
    bc = a2 * (W2 @ b1) + b2

    aup, bup = _bn_fold(f["fup_bn"])
    Wup = aup[:, None] * f["fup_w"].astype(np.float64)

    def enc_fold(w, bn):
        wts, bs = [], []
        for s in range(4):
            a, b = _bn_fold(bn[s])
            ws = a[:, None] * w[s].astype(np.float64)
            wts.append(ws.T.reshape(KC, P, C).astype(np.float32))
            bs.append(b.astype(np.float32))
        return np.stack(wts), np.stack(bs)[None]

    wxt, bex = enc_fold(f["enc_x_w"], f["enc_x_bn"])
    wyt, bey = enc_fold(f["enc_y_w"], f["enc_y_bn"])

    common = {
        "wct": Wc.T.reshape(KC, P, C).astype(np.float32),
        "wupt": Wup.T.reshape(KC, P, C).astype(np.float32),
        "bcd": bc.astype(np.float32).reshape(KC, P).T.copy(),
        "bupd": bup.astype(np.float32).reshape(KC, P).T.copy(),
        "wxt": wxt, "wyt": wyt, "bexd": bex, "beyd": bey,
        "lxtd": _split_lin(f["lin_x_w"]),
        "lytd": _split_lin(f["lin_y_w"]),
        "bxd": f["lin_x_b"].astype(np.float32).reshape(NPOOL, 1).copy(),
        "byd": f["lin_y_b"].astype(np.float32).reshape(NPOOL, 1).copy(),
    }

    in_maps = []
    for i in range(B):
        m = dict(common)
        m["xd"] = np.ascontiguousarray(
            f["x"][i].astype(np.float32).reshape(KC, P, HW))
        m["yd"] = np.ascontiguousarray(
            f["y"][i].astype(np.float32).reshape(KC, P, HW))
        in_maps.append(m)
    return in_maps


def _get_nc():
    global _NC
    if _NC is None:
        nc = bacc.Bacc("TRN2", target_bir_lowering=False)
        _NC = _emit(nc)
    return _NC


def _run(inputs, trace=False):
    nc = _get_nc()
    in_maps = _prep(inputs)
    res = run_bass_kernel_spmd(nc, in_maps, core_ids=list(range(B)),
                               trace=trace)
    out = np.empty((B, C, H, W), np.float32)
    for i in range(B):
        out[i] = res.results[i]["outd"].reshape(C, H, W)
    return out, res


def kernel(**inputs) -> np.ndarray:
    out, _ = _run(inputs, trace=False)
    return out


# revision 10
# speedup vs baseline: 1.2210x; 1.2210x over previous
"""Bass/Tile TRN2 kernel for nn_CPAMDec (CPAM cross-attention decoder).

Sharding: data-parallel over batch — 8 samples, one per NeuronCore.
All parameters are replicated; each core computes its full sample.

Host-side (parameter-only) preprocessing:
  - eval-mode BatchNorm affines folded into the adjacent 1x1-conv weights
  - the two chained fx convs fused into a single 512x512 matrix Wc

Key device-side algebra: both 512x512 convs over hw=5184 are eliminated
by reassociation through the 50-token attention bottleneck:
  sim  = (Wc@x + bc)^T @ fy  =  x^T @ G + const,   G = Wc^T @ fy [512,50]
  out  = Wup@(att@fself) + bup + x  =  (FW^T @ att^T) + bup + x,
         FW = fself @ Wup^T [50,512]
so the only per-pixel matmuls contract through 50 dims.

Per core (C=512 as 4 chunks of 128 partitions, hw=5184 as 12 tiles of
432 = 6 rows of 72):
  P0y: stream y tiles -> pool partials (DVE)
  P1y: finish y pooling, y encoder, fy, G = Wc^T@fy, const = fy^T@bc
  P2 : stream x tiles into a resident [512,5184] buffer; per tile:
       pool partials, simT = G^T@x_t (+const), PE-transpose, row softmax,
       att stored
  P1x: finish x pooling, x encoder, fself, FW = fself@Wup'^T
  P3 : per tile: PE-transpose att back, out = FW^T@attT + bup + x_t, DMA

The softmax-critical path (pool/enc/linear/G/sim) runs in exact fp32;
the post-softmax path (FW/out) runs in float32r.
"""

import sys

for _p in ("/opt/trn_rl_repo", "/root/.axon_site/_ro/trn_rl_repo"):
    if _p not in sys.path:
        sys.path.append(_p)

import numpy as np

import concourse.bacc as bacc
import concourse.bass as bass
import concourse.mybir as mybir
import concourse.tile as tile
from concourse.bass_utils import run_bass_kernel_spmd
from concourse.masks import make_identity

F32 = mybir.dt.float32
F32R = mybir.dt.float32r
AX = mybir.AxisListType
AF = mybir.ActivationFunctionType
ALU = mybir.AluOpType

B, C, H, W = 8, 512, 72, 72
HW = H * W            # 5184
KC, P = 4, 128        # channel chunks x partitions
NT, TW = 12, 432      # hw tiles: 12 x (6 rows of 72)
NSUB, SUB = 4, 108    # row-subblocks per tile for softmax
NPOOL = 50            # 1 + 4 + 9 + 36
EPS = 1e-5
S_OFF = (0, 1, 5, 14)
S_LEN = (1, 4, 9, 36)

_NC = None


def _emit(nc):
    xd = nc.dram_tensor("xd", [KC, P, HW], F32, kind="ExternalInput")
    yd = nc.dram_tensor("yd", [KC, P, HW], F32, kind="ExternalInput")
    wcd = nc.dram_tensor("wcd", [KC, P, C], F32, kind="ExternalInput")
    wupt = nc.dram_tensor("wupt", [KC, P, C], F32R, kind="ExternalInput")
    bcd = nc.dram_tensor("bcd", [P, KC], F32, kind="ExternalInput")
    bupd = nc.dram_tensor("bupd", [P, KC], F32, kind="ExternalInput")
    wxt = nc.dram_tensor("wxt", [4, KC, P, C], F32, kind="ExternalInput")
    wyt = nc.dram_tensor("wyt", [4, KC, P, C], F32, kind="ExternalInput")
    bexd = nc.dram_tensor("bexd", [1, 4, C], F32, kind="ExternalInput")
    beyd = nc.dram_tensor("beyd", [1, 4, C], F32, kind="ExternalInput")
    lxtd = nc.dram_tensor("lxtd", [4, S_LEN[3], NPOOL], F32,
                          kind="ExternalInput")
    lytd = nc.dram_tensor("lytd", [4, S_LEN[3], NPOOL], F32,
                          kind="ExternalInput")
    bxd = nc.dram_tensor("bxd", [NPOOL, 1], F32, kind="ExternalInput")
    byd = nc.dram_tensor("byd", [NPOOL, 1], F32, kind="ExternalInput")
    outd = nc.dram_tensor("outd", [KC, P, HW], F32, kind="ExternalOutput")

    with tile.TileContext(nc) as tc:
        _body(nc, tc, xd, yd, wcd, wupt, bcd, bupd, wxt, wyt, bexd, beyd,
              lxtd, lytd, bxd, byd, outd)
    nc.compile()
    return nc


def _body(nc, tc, xd, yd, wcd, wupt, bcd, bupd, wxt, wyt, bexd, beyd,
          lxtd, lytd, bxd, byd, outd):
    from contextlib import ExitStack
    ctx = ExitStack()
    with ctx:
        consts = ctx.enter_context(tc.tile_pool(name="consts", bufs=1))
        xresp = ctx.enter_context(tc.tile_pool(name="xresp", bufs=1))
        poolp = ctx.enter_context(tc.tile_pool(name="poolp", bufs=1))
        stream = ctx.enter_context(tc.tile_pool(name="stream", bufs=3))
        encp = ctx.enter_context(tc.tile_pool(name="encp", bufs=1))
        wenc = ctx.enter_context(tc.tile_pool(name="wenc", bufs=3))
        attp = ctx.enter_context(tc.tile_pool(name="attp", bufs=2))
        outp = ctx.enter_context(tc.tile_pool(name="outp", bufs=2))
        ps_sim = ctx.enter_context(
            tc.tile_pool(name="ps_sim", bufs=2, space="PSUM"))

        # ---- constants ----
        ident = consts.tile([P, P], F32)
        make_identity(nc, ident)
        wc_sb = consts.tile([P, KC, C], F32, tag="wc")
        nc.sync.dma_start(out=wc_sb, in_=wcd.ap().rearrange("k p m -> p k m"))
        wupt_sb = consts.tile([P, KC, C], F32R, tag="wupt")
        nc.sync.dma_start(out=wupt_sb, in_=wupt.ap().rearrange("k p m -> p k m"))
        bc_sb = consts.tile([P, KC], F32, tag="bc")
        nc.sync.dma_start(out=bc_sb, in_=bcd.ap())
        bup_sb = consts.tile([P, KC], F32, tag="bup")
        nc.sync.dma_start(out=bup_sb, in_=bupd.ap())
        lxt_sb = consts.tile([S_LEN[3], 4, NPOOL], F32, tag="lxt")
        nc.sync.dma_start(out=lxt_sb, in_=lxtd.ap().rearrange("s j k -> j s k"))
        lyt_sb = consts.tile([S_LEN[3], 4, NPOOL], F32, tag="lyt")
        nc.sync.dma_start(out=lyt_sb, in_=lytd.ap().rearrange("s j k -> j s k"))
        bx_sb = consts.tile([NPOOL, 1], F32, tag="bx")
        nc.sync.dma_start(out=bx_sb, in_=bxd.ap())
        by_sb = consts.tile([NPOOL, 1], F32, tag="by")
        nc.sync.dma_start(out=by_sb, in_=byd.ap())
        bex_sb = consts.tile([1, 4, C], F32, tag="bex")
        nc.sync.dma_start(out=bex_sb, in_=bexd.ap())
        bey_sb = consts.tile([1, 4, C], F32, tag="bey")
        nc.sync.dma_start(out=bey_sb, in_=beyd.ap())
        ones_sb = consts.tile([1, S_LEN[3]], F32, tag="ones")
        nc.vector.memset(ones_sb, 1.0)

        # ---- persistent buffers ----
        x_sb = xresp.tile([P, KC, HW], F32)
        partx = poolp.tile([P, KC, NT, 6], F32, tag="partx")
        party = poolp.tile([P, KC, NT, 6], F32, tag="party")
        att_store = poolp.tile([SUB, NT, NSUB, NPOOL], F32, tag="att_store")

        def pool_partial(t, xt, part):
            # block sums over (6 rows, 12 cols) -> [P, 6 w-blocks] per chunk
            for kc in range(KC):
                src = xt[:, kc, :].rearrange(
                    "p (lh wb wl) -> p wb lh wl", lh=6, wl=12)
                nc.vector.reduce_sum(part[:, kc, t, :], src, axis=AX.XY)

        def finish_pool(part, pooled):
            p6 = poolp.tile([P, KC, 6, 6], F32, tag="p6")
            s3 = poolp.tile([P, KC, 6, 3], F32, tag="s3")
            p3 = poolp.tile([P, KC, 3, 3], F32, tag="p3")
            s2 = poolp.tile([P, KC, 6, 2], F32, tag="s2")
            p2 = poolp.tile([P, KC, 2, 2], F32, tag="p2")
            p1 = poolp.tile([P, KC, 1], F32, tag="p1")
            for kc in range(KC):
                nc.vector.reduce_sum(
                    p6[:, kc], part[:, kc].rearrange(
                        "p (hh half) wb -> p hh wb half", half=2), axis=AX.X)
                nc.vector.reduce_sum(
                    s3[:, kc], p6[:, kc].rearrange(
                        "p hh (w3 wl) -> p hh w3 wl", wl=2), axis=AX.X)
                nc.vector.reduce_sum(
                    p3[:, kc], s3[:, kc].rearrange(
                        "p (h3 hl) w3 -> p h3 w3 hl", hl=2), axis=AX.X)
                nc.vector.reduce_sum(
                    s2[:, kc], p6[:, kc].rearrange(
                        "p hh (w2 wl) -> p hh w2 wl", wl=3), axis=AX.X)
                nc.vector.reduce_sum(
                    p2[:, kc], s2[:, kc].rearrange(
                        "p (h2 hl) w2 -> p h2 w2 hl", hl=3), axis=AX.X)
                nc.vector.reduce_sum(
                    p1[:, kc], p6[:, kc].rearrange("p a b -> p (a b)"),
                    axis=AX.X)
                nc.vector.tensor_scalar_mul(
                    pooled[:, kc, 0:1], p1[:, kc], 1.0 / 5184)
                nc.vector.tensor_scalar_mul(
                    pooled[:, kc, 1:5],
                    p2[:, kc].rearrange("p a b -> p (a b)"), 1.0 / 1296)
                nc.vector.tensor_scalar_mul(
                    pooled[:, kc, 5:14],
                    p3[:, kc].rearrange("p a b -> p (a b)"), 1.0 / 576)
                nc.vector.tensor_scalar_mul(
                    pooled[:, kc, 14:50],
                    p6[:, kc].rearrange("p a b -> p (a b)"), 1.0 / 144)

        def encoder(ps_pool, pooled, wt_dram, be_sb, enc_t):
            for s in range(4):
                off, ln = S_OFF[s], S_LEN[s]
                ep = ps_pool.tile([S_LEN[3], C], F32, tag="encps")
                for kc in range(KC):
                    wtile = wenc.tile([P, C], F32, tag="we")
                    nc.sync.dma_start(out=wtile, in_=wt_dram.ap()[s, kc])
                    nc.tensor.matmul(
                        ep[:ln, :], pooled[:, kc, off:off + ln], wtile,
                        start=(kc == 0), stop=False)
                nc.tensor.matmul(ep[:ln, :], ones_sb[:, :ln], be_sb[0:1, s, :],
                                 start=False, stop=True)
                nc.vector.tensor_scalar_max(enc_t[s][:ln, :], ep[:ln, :], 0.0)

        # ============ P0y: stream y, pool partials ============
        for t in range(NT):
            sl = slice(t * TW, (t + 1) * TW)
            yt = stream.tile([P, KC, TW], F32, tag="xt")
            nc.sync.dma_start(
                out=yt, in_=yd.ap()[:, :, sl].rearrange("k p n -> p k n"))
            pool_partial(t, yt, party)

        # ============ P1y: y pooling -> encoder -> fy -> G, const ==========
        pooledy = poolp.tile([P, KC, NPOOL], F32, tag="pooledy")
        finish_pool(party, pooledy)

        fy_sb = encp.tile([P, KC, NPOOL], F32, tag="fy")
        g_sb = encp.tile([P, KC, NPOOL], F32, tag="g")
        const_sb = encp.tile([NPOOL, 1], F32, tag="const")
        with tc.tile_pool(name="ps_1y", bufs=1, space="PSUM") as ps1:
            ency_t = [encp.tile([S_LEN[s], C], F32, tag=f"ency{s}",
                                name=f"ency{s}") for s in range(4)]
            encoder(ps1, pooledy, wyt, bey_sb, ency_t)

            fyt2 = encp.tile([NPOOL, C], F32, tag="fyt2")
            fp2 = ps1.tile([NPOOL, C], F32, tag="encps")
            for s in range(4):
                nc.tensor.matmul(fp2, lyt_sb[:S_LEN[s], s, :], ency_t[s],
                                 start=(s == 0), stop=(s == 3))
            nc.scalar.activation(fyt2, fp2, AF.Identity, bias=by_sb, scale=1.0)

            for mc in range(KC):
                tp = ps1.tile([P, NPOOL], F32, tag="fybt")
                nc.tensor.transpose(tp, fyt2[:, mc * P:(mc + 1) * P],
                                    ident[:NPOOL, :NPOOL])
                nc.vector.tensor_copy(fy_sb[:, mc, :], tp)

            # G = Wc^T @ fy  [c_in(4xP), 50]
            for mc in range(KC):
                gp = ps1.tile([P, NPOOL], F32, tag="fybt")
                for kc in range(KC):
                    nc.tensor.matmul(
                        gp, wc_sb[:, kc, mc * P:(mc + 1) * P], fy_sb[:, kc, :],
                        start=(kc == 0), stop=(kc == KC - 1))
                nc.vector.tensor_copy(g_sb[:, mc, :], gp)

            # const = fy^T @ bc  [50, 1]
            cp = ps1.tile([NPOOL, 1], F32, tag="constps")
            for kc in range(KC):
                nc.tensor.matmul(cp, fy_sb[:, kc, :], bc_sb[:, kc:kc + 1],
                                 start=(kc == 0), stop=(kc == KC - 1))
            nc.vector.tensor_copy(const_sb, cp)

        # ============ P2: stream x resident; per tile sim + softmax ========
        for t in range(NT):
            sl = slice(t * TW, (t + 1) * TW)
            nc.sync.dma_start(
                out=x_sb[:, :, sl],
                in_=xd.ap()[:, :, sl].rearrange("k p n -> p k n"))
            pool_partial(t, x_sb[:, :, sl], partx)

            sp = ps_sim.tile([NPOOL, TW], F32, tag="simp")
            for kc in range(KC):
                nc.tensor.matmul(sp, g_sb[:, kc, :], x_sb[:, kc, sl],
                                 start=(kc == 0), stop=(kc == KC - 1))
            simT_t = attp.tile([NPOOL, TW], F32, tag="simT")
            nc.scalar.activation(simT_t, sp, AF.Identity, bias=const_sb,
                                 scale=1.0)

            rp = ps_sim.tile([SUB, NSUB, NPOOL], F32, tag="strp")
            for j in range(NSUB):
                nc.tensor.transpose(rp[:, j, :],
                                    simT_t[:, j * SUB:(j + 1) * SUB],
                                    ident[:NPOOL, :NPOOL])
            negmax = attp.tile([SUB, NSUB], F32, tag="negmax")
            sumexp = attp.tile([SUB, NSUB], F32, tag="sumexp")
            rec = attp.tile([SUB, NSUB], F32, tag="rec")
            for j in range(NSUB):
                nc.vector.reduce_max(negmax[:, j:j + 1], rp[:, j, :],
                                     axis=AX.X, negate=True)
                nc.scalar.activation(att_store[:, t, j, :], rp[:, j, :],
                                     AF.Exp, bias=negmax[:, j:j + 1],
                                     scale=1.0, accum_out=sumexp[:, j:j + 1])
            nc.vector.reciprocal(rec, sumexp)
            for j in range(NSUB):
                nc.vector.tensor_scalar_mul(
                    att_store[:, t, j, :], att_store[:, t, j, :],
                    rec[:, j:j + 1])

        # ============ P1x: x pooling -> encoder -> fself -> FW ============
        pooledx = poolp.tile([P, KC, NPOOL], F32, tag="pooledx")
        finish_pool(partx, pooledx)

        fw_sb = encp.tile([NPOOL, C], F32R, tag="fw")
        with tc.tile_pool(name="ps_1x", bufs=1, space="PSUM") as ps1x:
            encx_t = [encp.tile([S_LEN[s], C], F32, tag=f"encx{s}",
                                name=f"encx{s}") for s in range(4)]
            encoder(ps1x, pooledx, wxt, bex_sb, encx_t)

            fselfT = encp.tile([NPOOL, C], F32, tag="fselfT")
            fp = ps1x.tile([NPOOL, C], F32, tag="encps")
            for s in range(4):
                nc.tensor.matmul(fp, lxt_sb[:S_LEN[s], s, :], encx_t[s],
                                 start=(s == 0), stop=(s == 3))
            nc.scalar.activation(fselfT, fp, AF.Identity, bias=bx_sb,
                                 scale=1.0)

            # fself_c = fselfT transposed to [c, 50] (f32r for FW matmul)
            fself_c = encp.tile([P, KC, NPOOL], F32R, tag="fself_c")
            for mc in range(KC):
                tp2 = ps1x.tile([P, NPOOL], F32, tag="fybt")
                nc.tensor.transpose(tp2, fselfT[:, mc * P:(mc + 1) * P],
                                    ident[:NPOOL, :NPOOL])
                nc.vector.tensor_copy(fself_c[:, mc, :], tp2)

            # FW = fself @ Wup'^T  [50, C]
            fwp = ps1x.tile([NPOOL, C], F32, tag="encps")
            for kc in range(KC):
                nc.tensor.matmul(fwp, fself_c[:, kc, :], wupt_sb[:, kc, :],
                                 start=(kc == 0), stop=(kc == KC - 1))
            nc.vector.tensor_copy(fw_sb, fwp)

        # ============ P3: out = FW^T @ attT + bup + x ============
        with tc.tile_pool(name="ps_out", bufs=2, space="PSUM") as pso, \
             tc.tile_pool(name="ps_att", bufs=2, space="PSUM") as psa:
            for t in range(NT):
                sl = slice(t * TW, (t + 1) * TW)
                ap_ = psa.tile([NPOOL, TW], F32, tag="attps")
                for j in range(NSUB):
                    nc.tensor.transpose(ap_[:, j * SUB:(j + 1) * SUB],
                                        att_store[:, t, j, :],
                                        ident[:SUB, :SUB])
                attT_t = attp.tile([NPOOL, TW], F32R, tag="attT")
                nc.vector.tensor_copy(attT_t, ap_)

                out_t = outp.tile([P, KC, TW], F32, tag="out")
                for mc in range(KC):
                    op_ = pso.tile([P, TW], F32, tag="outps")
                    nc.tensor.matmul(op_, fw_sb[:, mc * P:(mc + 1) * P],
                                     attT_t, start=True, stop=True)
                    nc.scalar.activation(out_t[:, mc, :], op_, AF.Identity,
                                         bias=bup_sb[:, mc:mc + 1], scale=1.0)
                    nc.vector.tensor_tensor(out_t[:, mc, :], out_t[:, mc, :],
                                            x_sb[:, mc, sl], ALU.add)
                nc.sync.dma_start(
                    out=outd.ap()[:, :, sl].rearrange("k p n -> p k n"),
                    in_=out_t)


def _split_lin(lw):
    # lin weight [50,50]; lhsT rows j split by pool scale -> [4, 36, 50]
    lt = lw.T.astype(np.float32)  # [j, k]
    out = np.zeros((4, S_LEN[3], NPOOL), np.float32)
    for s in range(4):
        out[s, :S_LEN[s]] = lt[S_OFF[s]:S_OFF[s] + S_LEN[s]]
    return out


def _bn_fold(bn):
    g, bt, m, v = [a.astype(np.float64) for a in bn]
    a = g / np.sqrt(v + EPS)
    return a, bt.astype(np.float64) - a * m


def _prep(inputs):
    """Host-side fold + shard. Returns list of 8 per-core input maps."""
    f = {k: np.asarray(v) for k, v in inputs.items()}

    a1, b1 = _bn_fold(f["fx_bn"][0])
    a2, b2 = _bn_fold(f["fx_bn"][1])
    W1 = f["fx_w"][0].astype(np.float64)
    W2 = f["fx_w"][1].astype(np.float64)
    Wc = (a2[:, None] * W2) @ (a1[:, None] * W1)
    bc = a2 * (W2 @ b1) + b2

    aup, bup = _bn_fold(f["fup_bn"])
    Wup = aup[:, None] * f["fup_w"].astype(np.float64)

    def enc_fold(w, bn):
        wts, bs = [], []
        for s in range(4):
            a, b = _bn_fold(bn[s])
            ws = a[:, None] * w[s].astype(np.float64)
            wts.append(ws.T.reshape(KC, P, C).astype(np.float32))
            bs.append(b.astype(np.float32))
        return np.stack(wts), np.stack(bs)[None]

    wxt, bex = enc_fold(f["enc_x_w"], f["enc_x_bn"])
    wyt, bey = enc_fold(f["enc_y_w"], f["enc_y_bn"])

    common = {
        "wcd": Wc.astype(np.float32).reshape(KC, P, C),
        "wupt": np.ascontiguousarray(
            Wup.T).astype(np.float32).reshape(KC, P, C),
        "bcd": bc.astype(np.float32).reshape(KC, P).T.copy(),
        "bupd": bup.astype(np.float32).reshape(KC, P).T.copy(),
        "wxt": wxt, "wyt": wyt, "bexd": bex, "beyd": bey,
        "lxtd": _split_lin(f["lin_x_w"]),
        "lytd": _split_lin(f["lin_y_w"]),
        "bxd": f["lin_x_b"].astype(np.float32).reshape(NPOOL, 1).copy(),
        "byd": f["lin_y_b"].astype(np.float32).reshape(NPOOL, 1).copy(),
    }

    in_maps = []
    for i in range(B):
        m = dict(common)
        m["xd"] = np.ascontiguousarray(
            f["x"][i].astype(np.float32).reshape(KC, P, HW))
        m["yd"] = np.ascontiguousarray(
            f["y"][i].astype(np.float32).reshape(KC, P, HW))
        in_maps.append(m)
    return in_maps


def _get_nc():
    global _NC
    if _NC is None:
        nc = bacc.Bacc("TRN2", target_bir_lowering=False)
        _NC = _emit(nc)
    return _NC


def _run(inputs, trace=False):
    nc = _get_nc()
    in_maps = _prep(inputs)
    res = run_bass_kernel_spmd(nc, in_maps, core_ids=list(range(B)),
                               trace=trace)
    out = np.empty((B, C, H, W), np.float32)
    for i in range(B):
        out[i] = res.results[i]["outd"].reshape(C, H, W)
    return out, res


def kernel(**inputs) -> np.ndarray:
    out, _ = _run(inputs, trace=False)
    return out
